# revision 23
# baseline (speedup 1.0000x reference)
"""Cross-attention SPMD Bass kernel for Trainium2 (8 NeuronCores).

Strategy: pure data-parallel over batch (b=16 -> 2 batch items per core).
All on-chip work is done in a "transposed attention" orientation:
  - attT[j, i] = (K^T blocks).T @ Q^T tiles   (j = seq on partitions)
  - softmax sums over j via a ones matmul on the PE (partition-dim reduce)
  - O^T[e, i] = V[j,e] blocks as lhsT @ EtT   (contraction over j)
  - OUT[c, i] = WoutT.T @ O^T                 (natural [c, hw] output layout)
The pad mask is fed pre-transposed from the host ([b, seq, hw] u8) and the
(huge) attention output is written as attT scratch [b, seq, hw]; the host
returns a transposed view. proj_in is folded into the Q projection
(Q = (Wq@W_in) x + (Wq b_in + bq)) - an algebraic fusion of two back-to-back
linear layers. All matmuls run in bf16 (casts are folded into the mandatory
PSUM->SBUF evacuations; fp32 accumulation in PSUM).

The block loop is software-pipelined two levels deep so the PE queue never
head-of-line blocks on the exp/mask chain: group jg's S/OT matmuls are
emitted after group jg+1's QK+exp, and each block's last group + softmax
reciprocal + O^T normalize + output projection + att staging are emitted
inside the NEXT block's loop.
"""

import numpy as np
import ml_dtypes

B = 16
NCORES = 8
BPC = B // NCORES  # batches per core
P = 128
EMB = 256
CH = 256
HW = 2304  # 48*48
SEQ = 2048
D = 128  # ctx feature dim
SCALE = float(EMB) ** -0.5

# i-blocks over hw dim: 4x512 + 256
IBLKS = [(0, 512), (512, 512), (1024, 512), (1536, 512), (2048, 256)]
NJC = SEQ // P  # 16 j-chunks
NJG = NJC // 4  # groups of 4 j-chunks


def _build_program():
    import concourse.bass as bass
    import concourse.tile as tile
    import concourse.mybir as mybir
    from concourse import bacc
    from contextlib import ExitStack

    f32 = mybir.dt.float32
    bf16 = mybir.dt.bfloat16
    u8 = mybir.dt.uint8
    Alu = mybir.AluOpType
    Act = mybir.ActivationFunctionType

    nc = bacc.Bacc(
        "TRN2",
        target_bir_lowering=False,
        debug=False,
        num_devices=NCORES,
    )

    # ---- DRAM I/O ----
    x_d = nc.dram_tensor("x", [BPC, CH, HW], bf16, kind="ExternalInput").ap()
    ctxT_d = nc.dram_tensor("ctxT", [BPC, D, SEQ], bf16, kind="ExternalInput").ap()
    maskT_d = nc.dram_tensor("maskT", [BPC, SEQ, HW], u8, kind="ExternalInput").ap()
    WqinT_d = nc.dram_tensor("WqinT", [CH, EMB], bf16, kind="ExternalInput").ap()
    WkT_d = nc.dram_tensor("WkT", [D, EMB], bf16, kind="ExternalInput").ap()
    WvT_d = nc.dram_tensor("WvT", [D, EMB], bf16, kind="ExternalInput").ap()
    WoutT_d = nc.dram_tensor("WoutT", [EMB, CH], bf16, kind="ExternalInput").ap()
    bqin_d = nc.dram_tensor("b_qin", [EMB], f32, kind="ExternalInput").ap()
    bk_d = nc.dram_tensor("b_k", [EMB], f32, kind="ExternalInput").ap()
    bv_d = nc.dram_tensor("b_v", [EMB], f32, kind="ExternalInput").ap()
    bout_d = nc.dram_tensor("b_out", [CH], f32, kind="ExternalInput").ap()

    attT_d = nc.dram_tensor("attT", [BPC, SEQ, HW], f32, kind="ExternalOutput").ap()
    out_d = nc.dram_tensor("out", [BPC, CH, HW], f32, kind="ExternalOutput").ap()

    with tile.TileContext(nc) as tc, ExitStack() as ctx:
        wp = ctx.enter_context(tc.tile_pool(name="weights", bufs=1))
        perb = ctx.enter_context(tc.tile_pool(name="perbatch", bufs=2))
        xs = ctx.enter_context(tc.tile_pool(name="xstream", bufs=3))
        etp = ctx.enter_context(tc.tile_pool(name="etpool", bufs=2))
        otp = ctx.enter_context(tc.tile_pool(name="otpool", bufs=2))
        mp = ctx.enter_context(tc.tile_pool(name="maskpool", bufs=5))
        asg = ctx.enter_context(tc.tile_pool(name="attstage", bufs=3))
        osg = ctx.enter_context(tc.tile_pool(name="outstage", bufs=4))
        sm = ctx.enter_context(tc.tile_pool(name="small", bufs=4))
        ps_a = ctx.enter_context(tc.tile_pool(name="ps_a", bufs=2, space="PSUM"))
        ps_ot = ctx.enter_context(tc.tile_pool(name="ps_ot", bufs=2, space="PSUM"))
        ps_s = ctx.enter_context(tc.tile_pool(name="ps_s", bufs=2, space="PSUM"))

        # ---- constants / weights to SBUF ----
        WqinT_sb = wp.tile([P, 2, EMB], bf16)
        nc.sync.dma_start(WqinT_sb, WqinT_d.rearrange("(co p) e -> p co e", p=P))
        WkT_sb = wp.tile([P, EMB], bf16)
        nc.sync.dma_start(WkT_sb, WkT_d)
        WvT_sb = wp.tile([P, EMB], bf16)
        nc.sync.dma_start(WvT_sb, WvT_d)
        WoutT_sb = wp.tile([P, 2, CH], bf16)
        nc.sync.dma_start(WoutT_sb, WoutT_d.rearrange("(eo p) c -> p eo c", p=P))

        bqin_sb = wp.tile([P, 2], f32)
        nc.sync.dma_start(bqin_sb, bqin_d.rearrange("(eo p) -> p eo", p=P))
        bk_sb = wp.tile([P, 2], f32)
        nc.sync.dma_start(bk_sb, bk_d.rearrange("(eo p) -> p eo", p=P))
        bout_sb = wp.tile([P, 2], f32)
        nc.sync.dma_start(bout_sb, bout_d.rearrange("(co p) -> p co", p=P))
        bv_sb = wp.tile([P, EMB], f32)
        nc.gpsimd.dma_start(bv_sb, bv_d[None, :].to_broadcast((P, EMB)))

        ones_sb = wp.tile([P, P], bf16)
        nc.vector.memset(ones_sb, 1.0)

        # ---------- pipelined block helpers ----------
        def qk_exp_group(st, jg):
            bi, i0, iw, Et = st["bi"], st["i0"], st["iw"], st["Et"]
            m_t = mp.tile([P, 4, 512], u8, tag="mask")
            nc.sync.dma_start(
                m_t[:, :, :iw],
                maskT_d[
                    bi, jg * 4 * P : (jg + 1) * 4 * P, i0 : i0 + iw
                ].rearrange("(g p) i -> p g i", p=P),
            )
            st["m"][jg] = m_t
            for g in range(4):
                jc = jg * 4 + g
                a_ps = ps_a.tile([P, 512], f32, tag="ps_a")
                for eo in range(2):
                    nc.tensor.matmul(
                        a_ps[:, :iw],
                        lhsT=st["KT"][:, eo, jc * P : (jc + 1) * P],
                        rhs=st["QT"][:, eo, i0 : i0 + iw],
                        start=(eo == 0),
                        stop=(eo == 1),
                    )
                nc.scalar.activation(
                    out=Et[:, jc, :iw],
                    in_=a_ps[:, :iw],
                    func=Act.Exp,
                    scale=SCALE,
                )

        def sot_group(st, jg):
            iw, Et = st["iw"], st["Et"]
            # Et *= (mask == 0), one fused op per 4-chunk group
            nc.vector.scalar_tensor_tensor(
                out=Et[:, jg * 4 : (jg + 1) * 4, :iw],
                in0=st["m"][jg][:, :, :iw],
                scalar=0.0,
                in1=Et[:, jg * 4 : (jg + 1) * 4, :iw],
                op0=Alu.is_equal,
                op1=Alu.mult,
            )
            for g in range(4):
                jc = jg * 4 + g
                nc.tensor.matmul(
                    st["S_ps"][:, :iw],
                    lhsT=ones_sb,
                    rhs=Et[:, jc, :iw],
                    start=(jc == 0),
                    stop=(jc == NJC - 1),
                )
                for eo in range(2):
                    nc.tensor.matmul(
                        st["OT_ps"][:, eo, :iw],
                        lhsT=st["V"][:, jc, eo * P : (eo + 1) * P],
                        rhs=Et[:, jc, :iw],
                        start=(jc == 0),
                        stop=(jc == NJC - 1),
                    )

        def finish_block(st):
            iw = st["iw"]
            sot_group(st, NJG - 1)
            recipS = sm.tile([P, 512], f32, tag="recip")
            nc.vector.reciprocal_approx_fast(recipS[:, :iw], st["S_ps"][:, :iw])
            st["recip"] = recipS
            OT_sb = otp.tile([P, 2, 512], bf16, tag="ot")
            for eo in range(2):
                nc.vector.tensor_tensor(
                    out=OT_sb[:, eo, :iw],
                    in0=st["OT_ps"][:, eo, :iw],
                    in1=recipS[:, :iw],
                    op=Alu.mult,
                )
            st["OT_sb"] = OT_sb

        def att_out_group(st, jg):
            i0, iw, bi = st["i0"], st["iw"], st["bi"]
            a_t = asg.tile([P, 4, 512], f32, tag="attstage")
            eng = nc.gpsimd if jg % 2 == 1 else nc.vector
            eng.tensor_tensor(
                out=a_t[:, :, :iw],
                in0=st["Et"][:, jg * 4 : (jg + 1) * 4, :iw],
                in1=st["recip"][:, None, :iw].to_broadcast((P, 4, iw)),
                op=Alu.mult,
            )
            nc.sync.dma_start(
                attT_d[
                    bi, jg * 4 * P : (jg + 1) * 4 * P, i0 : i0 + iw
                ].rearrange("(g p) i -> p g i", p=P),
                a_t[:, :, :iw],
            )

        def out_proj(st):
            i0, iw, bi = st["i0"], st["iw"], st["bi"]
            out_re = out_d[bi].rearrange("(co p) i -> p co i", p=P)
            for co in range(2):
                o_ps = ps_a.tile([P, 512], f32, tag="ps_a")
                for eo in range(2):
                    nc.tensor.matmul(
                        o_ps[:, :iw],
                        lhsT=WoutT_sb[:, eo, co * P : (co + 1) * P],
                        rhs=st["OT_sb"][:, eo, :iw],
                        start=(eo == 0),
                        stop=(eo == 1),
                    )
                out_t = osg.tile([P, 512], f32, tag="outstage")
                nc.scalar.activation(
                    out=out_t[:, :iw],
                    in_=o_ps[:, :iw],
                    func=Act.Identity,
                    bias=bout_sb[:, co : co + 1],
                    scale=1.0,
                )
                nc.sync.dma_start(out_re[:, co, i0 : i0 + iw], out_t[:, :iw])

        carry = None  # block in flight (last group + epilogue pending)
        done = None  # block whose epilogue ops are emitted, outputs pending

        for bi in range(BPC):
            # ================= projections =================
            ctx_sb = perb.tile([P, SEQ], bf16, tag="ctx")
            nc.sync.dma_start(ctx_sb, ctxT_d[bi])

            QT_sb = perb.tile([P, 2, HW], bf16, tag="qt")
            x_re = x_d[bi].rearrange("(co p) i -> p co i", p=P)
            for i0, iw in IBLKS:
                x_t = xs.tile([P, 2, 512], bf16, tag="xs")
                nc.sync.dma_start(x_t[:, :, :iw], x_re[:, :, i0 : i0 + iw])
                for eo in range(2):
                    ps = ps_a.tile([P, 512], f32, tag="ps_a")
                    for co in range(2):
                        nc.tensor.matmul(
                            ps[:, :iw],
                            lhsT=WqinT_sb[:, co, eo * P : (eo + 1) * P],
                            rhs=x_t[:, co, :iw],
                            start=(co == 0),
                            stop=(co == 1),
                        )
                    nc.scalar.activation(
                        out=QT_sb[:, eo, i0 : i0 + iw],
                        in_=ps[:, :iw],
                        func=Act.Identity,
                        bias=bqin_sb[:, eo : eo + 1],
                        scale=1.0,
                    )

            KT_sb = perb.tile([P, 2, SEQ], bf16, tag="kt")
            for jt in range(SEQ // 512):
                for eo in range(2):
                    ps = ps_a.tile([P, 512], f32, tag="ps_a")
                    nc.tensor.matmul(
                        ps,
                        lhsT=WkT_sb[:, eo * P : (eo + 1) * P],
                        rhs=ctx_sb[:, jt * 512 : (jt + 1) * 512],
                        start=True,
                        stop=True,
                    )
                    nc.scalar.activation(
                        out=KT_sb[:, eo, jt * 512 : (jt + 1) * 512],
                        in_=ps,
                        func=Act.Identity,
                        bias=bk_sb[:, eo : eo + 1],
                        scale=1.0,
                    )

            V_sb = perb.tile([P, NJC, EMB], bf16, tag="v")
            for jc in range(NJC):
                ps = ps_a.tile([P, 512], f32, tag="ps_a")
                nc.tensor.matmul(
                    ps[:, :EMB],
                    lhsT=ctx_sb[:, jc * P : (jc + 1) * P],
                    rhs=WvT_sb,
                    start=True,
                    stop=True,
                )
                nc.vector.tensor_tensor(
                    out=V_sb[:, jc, :],
                    in0=ps[:, :EMB],
                    in1=bv_sb,
                    op=Alu.add,
                )

            # ================= attention, two-level pipelined =================
            for i0, iw in IBLKS:
                st = {
                    "bi": bi,
                    "i0": i0,
                    "iw": iw,
                    "Et": etp.tile([P, NJC, 512], bf16, tag="et", name="Et"),
                    "S_ps": ps_s.tile([P, 512], f32, tag="ps_s", name="S_ps"),
                    "OT_ps": ps_ot.tile([P, 2, 512], f32, tag="ps_ot", name="OT_ps"),
                    "QT": QT_sb,
                    "KT": KT_sb,
                    "V": V_sb,
                    "m": {},
                }
                qk_exp_group(st, 0)
                if carry is not None:
                    finish_block(carry)
                    done = carry
                for jg in range(1, NJG):
                    qk_exp_group(st, jg)
                    sot_group(st, jg - 1)
                    if done is not None:
                        if jg == 2:
                            out_proj(done)
                        att_out_group(done, jg - 1)
                if done is not None:
                    att_out_group(done, NJG - 1)
                carry = st

        finish_block(carry)
        out_proj(carry)
        for jg in range(NJG):
            att_out_group(carry, jg)

    nc.compile()
    return nc


_prog_cache = {}


def _get_program():
    if "nc" not in _prog_cache:
        _prog_cache["nc"] = _build_program()
    return _prog_cache["nc"]


def make_in_maps(x, context, pad_mask, W_in, b_in, Wq, bq, Wk, bk, Wv, bv, W_out, b_out):
    bf = ml_dtypes.bfloat16
    Wqin = np.asarray(Wq, np.float32) @ np.asarray(W_in, np.float32)
    b_qin = np.asarray(Wq, np.float32) @ np.asarray(b_in, np.float32) + np.asarray(
        bq, np.float32
    )
    shared = {
        "WqinT": np.ascontiguousarray(Wqin.T).astype(bf),
        "WkT": np.ascontiguousarray(Wk.T).astype(bf),
        "WvT": np.ascontiguousarray(Wv.T).astype(bf),
        "WoutT": np.ascontiguousarray(W_out.T).astype(bf),
        "b_qin": b_qin,
        "b_k": np.asarray(bk, dtype=np.float32),
        "b_v": np.asarray(bv, dtype=np.float32),
        "b_out": np.asarray(b_out, dtype=np.float32),
    }
    in_maps = []
    for c in range(NCORES):
        sl = slice(BPC * c, BPC * (c + 1))
        in_maps.append(
            {
                "x": np.asarray(x[sl], dtype=np.float32)
                .reshape(BPC, CH, HW)
                .astype(bf),
                "ctxT": np.ascontiguousarray(
                    np.asarray(context[sl], dtype=np.float32).transpose(0, 2, 1)
                ).astype(bf),
                "maskT": np.ascontiguousarray(
                    np.asarray(pad_mask[sl]).transpose(0, 2, 1)
                ).view(np.uint8),
                **shared,
            }
        )
    return in_maps


def _run(inputs, **spmd_kwargs):
    from concourse.bass_utils import run_bass_kernel_spmd

    nc = _get_program()
    in_maps = make_in_maps(**{k: np.asarray(v) for k, v in inputs.items()})
    res = run_bass_kernel_spmd(nc, in_maps, core_ids=list(range(NCORES)), **spmd_kwargs)

    out = np.empty((B, CH, 48, 48), np.float32)
    att = np.empty((B, HW, SEQ), np.float32)
    for c in range(NCORES):
        r = res.results[c]
        sl = slice(BPC * c, BPC * (c + 1))
        out[sl] = np.asarray(r["out"], dtype=np.float32).reshape(BPC, CH, 48, 48)
        att[sl] = np.asarray(r["attT"], dtype=np.float32).transpose(0, 2, 1)
    return (out, att), res


def kernel(**inputs):
    (out, att), _ = _run(inputs)
    return out, att
